# revision 34
# baseline (speedup 1.0000x reference)
"""CRF decode (conv features -> emission scores -> Viterbi) on 8 TRN2 cores.

Data-parallel over the batch: each core gets B/8 = 4096 words (32 tiles of
128 words on partitions). Per core:

  - conv+emission collapse to one (128 -> 26) linear map A = W @ C applied on
    the PE per 128-word tile (batched: 7 transposes -> one PSUM->SBUF copy,
    14 matmuls into one PSUM bank -> one fused scale+round prep),

  - Viterbi forward DP on packed integers:
      TTKP[p,j,i] = OFF_T + 32*That32[i,j] + (25-i) - 15.5
      vall[p,i]   = OFF_V + 32*vhat[p,i]            (multiple of 32)
      P[p,j]      = max_i (TTKP + vall)             (carries argmax in low bits)
    One custom-DVE scan per (tile, step) computes P directly into persistent
    storage via a stride-0 output AP (page-final element wins), and ONE
    fused Pool scalar_tensor_tensor per (group, step) does the v-update:
      vall' = RNE32(P + 2^28) + s32n   (both adds exact in fp32)
    eliminating the 3-op unpack chain of the previous design,

  - a slice of tiles runs its max on the Pool engine instead (stt add +
    tensor_reduce over the same packed operands), shedding DVE scans to the
    otherwise idle Pool engine,

  - emissions/DP overlap via a staggered march of ramped tile-groups,

  - batched backtrack straight off the packed P storage (fused select+scan
    custom op per step + 3 tiny affine decodes, all on the DVE),

  - int32 convert + strided DMA out.
"""

import sys

if "/opt/trn_rl_repo" not in sys.path:
    sys.path.insert(0, "/opt/trn_rl_repo")

import numpy as np

import copy as _copy
from dataclasses import dataclass
from typing import Any

import concourse.bacc as bacc
import concourse.mybir as mybir
import concourse.tile as tile
from concourse import bass_utils
from concourse import dve_ops
from concourse.dve_ops import DveOp
from concourse.dve_spec import (
    Spec, Src0, Src1, C0, C1, Idx, SubIdx, eq, lower, scan,
)
from concourse.dve_uop import AluInp, AluOp as UAluOp, DveOpSpec, Trigger


_HAND_CACHE: dict = {}


@dataclass(frozen=True)
class _HandDveOp(DveOp):
    """DveOp whose compiled uops are post-processed by `patch` (adds the
    SUB_DIM_DONE step state that resets an in-body scan at page boundaries —
    not expressible in the Spec language)."""

    patch: Any = None

    def compile(self, ver):
        key = (self.name, ver)
        if (r := _HAND_CACHE.get(key)) is not None:
            return r
        uops = self.patch(lower(self.spec, ver=ver))
        for u in uops:
            u.validate(ver)
        res = DveOpSpec(
            name=self.name, opcode=dve_ops.get_dve_sub_opcode(self.name),
            uops=uops, rd1_en=True)
        _HAND_CACHE[key] = res
        return res


def _reset_scan_stage(dps):
    """In a copied steady datapath, turn the scan combine MAX(CURR, expr)
    into BYPASS(expr): the running max restarts from the current element."""
    for dp in dps:
        if (dp.op in (UAluOp.MAX, UAluOp.ADD)
                and dp.alu_src0 == AluInp.CURR_ALU_OUT):
            dp.op = UAluOp.BYPASS
            dp.alu_src0 = dp.alu_src1
            return True
    raise AssertionError("scan stage not found")


def _patch_segmax(uops):
    assert len(uops) == 2, uops
    seed, steady = uops
    steady.trigger = (Trigger.SRC_TENSOR_DONE, Trigger.SUB_DIM_DONE,
                      Trigger.NONE)
    steady.next_uop = (0, 2, 0)
    step = _copy.deepcopy(steady)
    step.trigger = (Trigger.SRC_TENSOR_DONE, Trigger.SUB_DIM_DONE,
                    Trigger.COUNT)
    step.next_uop = (0, 2, 1)
    step.repeat_count = 1
    _reset_scan_stage(step.datapath_config)
    return [seed, steady, step]


def _register_hand_op(name, spec, patch):
    if name in dve_ops._SUB_OPCODE_FOR_NAME:
        for op in dve_ops.OPS:
            if op.name == name:
                return op
    opcode = max(dve_ops._SUB_OPCODE_FOR_NAME.values()) + 1
    dve_ops._SUB_OPCODE_FOR_NAME[name] = opcode
    shas = {}
    for ver in ("v3", "v4"):
        uops = patch(lower(spec, ver=ver))
        s = DveOpSpec(name=name, opcode=opcode, uops=uops, rd1_en=True)
        shas[ver] = s.sha(ver)
    op = _HandDveOp(name, spec, True, shas, patch=patch)
    dve_ops.OPS.append(op)
    dve_ops.CUSTOM_DVE_SPECS[name] = spec
    return op


def _register_segmax2():
    """r[p,s,n] = running max over n' <= n (within page s) of
    (in0 + in1*s0)[p,s,n'].  Page-final slice [:, :, N-1] is the grouped
    max; with packed-integer operands it carries the argmax in the low bits."""

    def _ref(in0, in1, s0, s1, imm2):
        N = in0.shape[-1]
        P = in0.shape[0]
        a = (np.asarray(in0, np.float32).reshape(P, -1, N)
             + np.asarray(in1, np.float32).reshape(P, -1, N)
             * np.float32(s0))
        return np.maximum.accumulate(a, axis=2).reshape(in0.shape)

    spec = Spec(body=scan(UAluOp.MAX, Src0 + Src1 * C0), reference=_ref)
    return _register_hand_op("SEGMAX2_ANT", spec, _patch_segmax)


SEGMAX2 = _register_segmax2()


def _register_eqsel():
    """out[p,s,n] = (n == in1[p,s,n]) * in0[p,s,n] — one-hot select of a
    row by label index, one pass; max-reduce of the output gives the
    selected (positive) value."""
    name = "EQSEL_ANT"
    if name in dve_ops._SUB_OPCODE_FOR_NAME:
        for op in dve_ops.OPS:
            if op.name == name:
                return op

    def _ref(in0, in1, s0, s1, imm2):
        N = in0.shape[-1]
        P = in0.shape[0]
        a = np.asarray(in0, np.float32).reshape(P, -1, N)
        b = np.asarray(in1, np.float32).reshape(a.shape)
        S = a.shape[1]
        n = (np.arange(S * N, dtype=np.float32)
             - np.repeat(np.arange(S), N) * s1).reshape(S, N)
        return ((n[None] == b).astype(np.float32) * a).reshape(in0.shape)

    spec = Spec(body=eq(Idx - SubIdx * C1, Src1) * Src0, reference=_ref)
    opcode = max(dve_ops._SUB_OPCODE_FOR_NAME.values()) + 1
    dve_ops._SUB_OPCODE_FOR_NAME[name] = opcode
    shas = {}
    for ver in ("v3", "v4"):
        sp = DveOpSpec(name=name, opcode=opcode, uops=lower(spec, ver=ver),
                       rd1_en=True)
        shas[ver] = sp.sha(ver)
    op = DveOp(name, spec, subdim=True, uops_sha=shas)
    dve_ops.OPS.append(op)
    dve_ops.CUSTOM_DVE_SPECS[name] = spec
    return op


EQSEL = _register_eqsel()


def _patch_eqselmax(uops):
    """Fuse a running page-max onto the lowered EQSEL program: the steady
    uop's first BYPASS stage after the select becomes MAX(CURR, select);
    the page-step uop keeps its BYPASS there, which restarts the max at
    page boundaries (same state machine the SubIdx counter already uses)."""
    assert len(uops) == 3, uops
    steady, step = uops[1], uops[2]
    for u in (steady, step):
        dps = u.datapath_config
        last = max(i for i, dp in enumerate(dps) if dp.op != UAluOp.BYPASS)
        assert dps[last].op == UAluOp.MULTIPLY and last + 1 < len(dps), dps
    dp = uops[1].datapath_config[
        1 + max(i for i, d in enumerate(uops[1].datapath_config)
                if d.op != UAluOp.BYPASS)]
    dp.op = UAluOp.MAX
    dp.alu_src0 = AluInp.CURR_ALU_OUT
    dp.alu_src1 = AluInp.PREV_ALU_OUT
    return uops


def _register_eqselmax():
    """r[p,s,n] = running max over n' <= n (within page s) of
    (n' == in1[p,s,n']) * in0[p,s,n'].  Page-final slice is in0 at the
    selected index (in0 > 0 assumed): select + reduce in ONE pass."""

    def _ref(in0, in1, s0, s1, imm2):
        N = in0.shape[-1]
        P = in0.shape[0]
        a = np.asarray(in0, np.float32).reshape(P, -1, N)
        b = np.asarray(in1, np.float32).reshape(a.shape)
        S = a.shape[1]
        n = (np.arange(S * N, dtype=np.float32)
             - np.repeat(np.arange(S), N) * np.float32(s1)).reshape(S, N)
        sel = (n[None] == b).astype(np.float32) * a
        return np.maximum.accumulate(sel, axis=2).reshape(in0.shape)

    spec = Spec(body=eq(Idx - SubIdx * C1, Src1) * Src0, reference=_ref)
    return _register_hand_op("EQSELMAX_ANT", spec, _patch_eqselmax)


EQSELMAX = _register_eqselmax()


F32 = mybir.dt.float32
I32 = mybir.dt.int32
AX = mybir.AxisListType
OP = mybir.AluOpType
ACTF = mybir.ActivationFunctionType

B = 32768
M = 14
H, WD = 16, 8
F = 128
L = 26
KS = 5
NCORES = 8
BC = B // NCORES          # words per core
NT = BC // 128            # 128-word tiles per core (32)

SC = 1024.0               # integer value scale
OFF_T = float(2.0 ** 21)  # packed offsets: P = OFF_T+OFF_V+32*(..) stays
OFF_V = float(2.0 ** 21)  # in (2^21, 2^23) so halves are exact
CR = float(2.0 ** 28)     # RNE-to-multiple-of-32 magnitude (ulp 32 there)

GROUPS = [1, 1, 1, 1, 1, 2, 2, 3, 3, 4, 4, 4, 5]   # word-tiles per stagger slot
GMAX = max(GROUPS)
NG = len(GROUPS)
BASES = [sum(GROUPS[:i]) for i in range(NG)]

POOL_START = 32           # tiles >= this run their max on the Pool engine
RESET = -float(2.0 ** 30)  # page-reset magnitude for the Pool scan trick


def _conv_matrix(K: np.ndarray) -> np.ndarray:
    """C[o, i] such that conv_SAME(x.reshape(H,WD)) flattened == C @ x."""
    K2 = K.reshape(KS, KS).astype(np.float64)
    C = np.zeros((F, F), dtype=np.float64)
    for r in range(H):
        for c in range(WD):
            o = r * WD + c
            for dy in range(KS):
                for dx in range(KS):
                    rr = r + dy - KS // 2
                    cc = c + dx - KS // 2
                    if 0 <= rr < H and 0 <= cc < WD:
                        C[o, rr * WD + cc] = K2[dy, dx]
    return C


def _consts(X, K, b, W, T):
    """Host-side constant tensors (fp64 math, one final fp32 round)."""
    C = _conv_matrix(K)
    A = W.astype(np.float64) @ C                         # (L, F)
    c0 = float(b[0]) * W.astype(np.float64).sum(axis=1)  # (L,)
    Tp = T.astype(np.float64) + c0[None, :]              # T'[i,j] = T[i,j]+c0[j]
    AT = np.ascontiguousarray(A.T).astype(np.float32)    # (F, L)

    # drift estimate from a 256-word exact DP (keeps v centered so the
    # packed range stays well inside (0, 2^23))
    sample = np.ascontiguousarray(X[:256], np.float32)
    sc = (sample.reshape(256 * M, F) @ AT).astype(np.float64)
    sc = sc.reshape(256, M, L)
    v = sc[:, 0] + c0[None, :]
    v0m = v.mean()
    for t in range(1, M):
        v = (v[:, :, None] + Tp[None]).max(axis=1) + sc[:, t]
    d = (v.mean() - v0m) / (M - 1)

    # packed transition table: TTKP[p, j, i] = OFF_T + 32*round(T'[i,j]*SC)
    #                                          + (25 - i) - 15.5
    That = (OFF_T + 32.0 * np.round(Tp.T * SC)
            + (L - 1 - np.arange(L))[None, :] - 15.5)
    TTKP = np.broadcast_to(That.astype(np.float32)[None], (128, L, L)).copy()
    # padded variant for the Pool-engine scan: slot 0 of each 27-wide page
    # holds the RESET sentinel (paired with RST's -2^30 add, it restarts the
    # running max at page boundaries of the single flat tensor_tensor_scan)
    ThatP = np.full((L, L + 1), RESET, np.float64)
    ThatP[:, 1:] = That
    TTKPP = np.broadcast_to(ThatP.astype(np.float32)[None],
                            (128, L, L + 1)).copy()
    RST = np.zeros((128, L * (L + 1)), np.float32)
    RST[:, ::L + 1] = RESET
    # v-init constant: supplies OFF_V and cancels the prep's -CR - OFF_T
    c0pn = (OFF_V + 32.0 * np.round(c0 * SC) + CR + OFF_T).astype(np.float32)
    C0P = np.broadcast_to(c0pn[None], (128, L)).copy()
    # prep bias: psb*(32*SC) + BIASD lands near -2^28 (ulp 32) so the fp32
    # add rounds scores to exact multiples of 32
    biasd = np.full((128, 1), -32.0 * d * SC - CR - OFF_T, np.float32)
    IRJ = np.broadcast_to(
        ((L - 1 - np.arange(L)) - 15.5).astype(np.float32)[None],
        (128, L)).copy()
    IDN = np.eye(128, dtype=np.float32)
    return AT, TTKP, TTKPP, RST, C0P, biasd, IRJ, IDN


def build_module():
    nc = bacc.Bacc("TRN2", target_bir_lowering=False, debug=False,
                   num_devices=NCORES)
    xs = nc.dram_tensor("XS", [BC, M, F], F32, kind="ExternalInput")
    at_d = nc.dram_tensor("AT", [F, L], F32, kind="ExternalInput")
    ttkp_d = nc.dram_tensor("TTKP", [128, L, L], F32, kind="ExternalInput")
    ttkpp_d = nc.dram_tensor("TTKPP", [128, L, L + 1], F32,
                             kind="ExternalInput")
    rst_d = nc.dram_tensor("RST", [128, L * (L + 1)], F32,
                           kind="ExternalInput")
    c0p_d = nc.dram_tensor("C0P", [128, L], F32, kind="ExternalInput")
    bd_d = nc.dram_tensor("BIASD", [128, 1], F32, kind="ExternalInput")
    ir_d = nc.dram_tensor("IRJ", [128, L], F32, kind="ExternalInput")
    id_d = nc.dram_tensor("IDN", [128, 128], F32, kind="ExternalInput")
    out_d = nc.dram_tensor("OUT", [BC, M], I32, kind="ExternalOutput")

    with tile.TileContext(nc) as tc:
        with (
            tc.tile_pool(name="const", bufs=1) as cpool,
            tc.tile_pool(name="pers", bufs=1) as ppool,
            tc.tile_pool(name="xin", bufs=3) as xpool,
            tc.tile_pool(name="xts", bufs=4) as tpool,
            tc.tile_pool(name="qq", bufs=3) as qpool,
            tc.tile_pool(name="psa", bufs=2, space="PSUM") as psA,
            tc.tile_pool(name="psb", bufs=2, space="PSUM") as psB,
        ):
            at = cpool.tile([F, L], F32)
            ttkp = cpool.tile([128, L, L], F32)
            ttkpp = cpool.tile([128, L, L + 1], F32)
            rst = cpool.tile([128, L * (L + 1)], F32)
            c0p = cpool.tile([128, L], F32)
            biasd = cpool.tile([128, 1], F32)
            irj = cpool.tile([128, L], F32)
            idn = cpool.tile([128, 128], F32)
            nc.sync.dma_start(idn[:], id_d.ap())
            nc.scalar.dma_start(at[:], at_d.ap())
            nc.scalar.dma_start(biasd[:], bd_d.ap())
            nc.scalar.dma_start(ttkp[:], ttkp_d.ap())
            nc.scalar.dma_start(ttkpp[:], ttkpp_d.ap())
            nc.scalar.dma_start(rst[:], rst_d.ap())
            nc.scalar.dma_start(c0p[:], c0p_d.ap())
            nc.scalar.dma_start(irj[:], ir_d.ap())

            ps = ppool.tile([128, NT, M - 1, L], F32)   # packed page-finals
            # packed v (multiples of 32); slot 0 is a zero guard column so
            # the Pool scan's padded pages read [guard, v0..v25]
            vall = ppool.tile([128, NT, L + 1], F32)
            s32 = ppool.tile([128, NT, M, L], F32)      # prepped emissions
            path = ppool.tile([128, NT, M], F32)
            pfb = ppool.tile([128, NT], F32)            # selected P / scratch
            kkb = ppool.tile([128, NT], F32)
            t2b = ppool.tile([128, NT], F32)
            pi = ppool.tile([128, NT, M], I32)
            nc.gpsimd.memset(vall[:, :, 0], 0.0)

            xs_t = xs.ap().rearrange("(n p) m f -> n p (m f)", p=128)

            # PE pstate warm-up: ~3us of continuous dummy transposes during
            # the initial DMA wait, so tile 0's real transposes run at full
            # clock instead of PE_CYCLE_PSTATE_LOW (3.7x slower)
            pwarm = psA.tile([128, 128], F32, tag="pwarm")
            for _ in range(12):
                nc.tensor.transpose(pwarm[:], idn[:], idn[:])

            def emit_emission(g):
                base = BASES[g]
                for k in range(GROUPS[g]):
                    wt = base + k
                    xt = xpool.tile([128, M * F], F32, tag="xt")
                    psb = psB.tile([128, M, L], F32, tag="psb")
                    for h in range(2):
                        nc.sync.dma_start(
                            xt[:, h * 7 * F:(h + 1) * 7 * F],
                            xs_t[wt][:, h * 7 * F:(h + 1) * 7 * F])
                        psa = psA.tile([128, 7, 128], F32, tag="psa")
                        for j in range(7):
                            m = h * 7 + j
                            nc.tensor.transpose(
                                psa[:, j, :], xt[:, m * F:(m + 1) * F],
                                idn[:])
                        xh = tpool.tile([128, 7, 128], F32, tag="xts")
                        nc.scalar.activation(xh[:], psa[:], ACTF.Copy)
                        for j in range(7):
                            m = h * 7 + j
                            nc.tensor.matmul(
                                psb[:, m, :], xh[:, j, :], at[:])
                        # fused scale + round-to-multiple-of-32 prep
                        nc.scalar.activation(
                            s32[:, wt, h * 7:(h + 1) * 7, :],
                            psb[:, h * 7:(h + 1) * 7, :], ACTF.Identity,
                            scale=32.0 * SC, bias=biasd[:, 0:1])

            def emit_init(g):
                base, G = BASES[g], GROUPS[g]
                nc.gpsimd.tensor_tensor(
                    vall[:, base:base + G, 1:], s32[:, base:base + G, 0, :],
                    c0p[:].unsqueeze(1).broadcast_to((128, G, L)), op=OP.add)

            def emit_step(g, t):
                base, G = BASES[g], GROUPS[g]
                for k in range(G):
                    wt = base + k
                    pslot = ps[:, wt, t - 1, :]
                    if wt >= POOL_START:
                        # Pool path: materialize q = TTKPP + v over padded
                        # 27-wide pages, then one flat tensor_tensor_scan
                        # whose RST vector restarts the running max at each
                        # page's RESET slot; page-finals land at [:, :, 26]
                        vb = vall[:, wt, :].unsqueeze(1).broadcast_to(
                            (128, L, L + 1))
                        q = qpool.tile([128, L, L + 1], F32, tag="q")
                        nc.gpsimd.tensor_tensor(
                            q[:], ttkpp[:], vb, op=OP.add)
                        nc.gpsimd.tensor_tensor_scan(
                            q[:].rearrange("p a b -> p (a b)"),
                            q[:].rearrange("p a b -> p (a b)"), rst[:],
                            0.0, op0=OP.max, op1=OP.add)
                        nc.gpsimd.tensor_copy(pslot, q[:, :, L])
                        kq = qpool.tile([128, L], F32, tag="kq")
                        nc.gpsimd.tensor_scalar(
                            kq[:], q[:, :, L], 1.0, CR, op0=OP.mult,
                            op1=OP.add)
                        nc.gpsimd.tensor_tensor(
                            vall[:, wt, 1:], kq[:], s32[:, wt, t, :],
                            op=OP.add)
                    else:
                        # stride-0 out: all 26 writes of a page land on the
                        # same address; the page-final (the grouped max) wins
                        vb = vall[:, wt, 1:].unsqueeze(1).broadcast_to(
                            (128, L, L))
                        pout = pslot.unsqueeze(2).broadcast_to((128, L, L))
                        nc.vector._custom_dve(
                            SEGMAX2, out=pout, in0=ttkp[:], in1=vb, s0=1.0)
                # v-update for the DVE tiles of this group:
                # vall = RNE32(P + 2^28) + s32[t]   (both adds exact)
                dhi = min(base + G, POOL_START)
                if dhi > base:
                    kk = qpool.tile([128, GMAX, L], F32, tag="kk")
                    nc.gpsimd.tensor_scalar(
                        kk[:, :dhi - base], ps[:, base:dhi, t - 1, :],
                        1.0, CR, op0=OP.mult, op1=OP.add)
                    nc.gpsimd.tensor_tensor(
                        vall[:, base:dhi, 1:], kk[:, :dhi - base],
                        s32[:, base:dhi, t, :], op=OP.add)

            emit_emission(0)
            emit_emission(1)
            for r in range(M - 1 + NG):
                for g in range(NG):
                    t = r - g
                    if t < 0 or t > M - 1:
                        continue
                    if t == 0:
                        emit_init(g)
                        if g + 2 < NG:
                            emit_emission(g + 2)
                    else:
                        emit_step(g, t)

            # final label: packed argmax of vall + IRJ in one scan pass
            irb = irj[:].unsqueeze(1).broadcast_to((128, NT, L))
            pfo = pfb[:].unsqueeze(2).broadcast_to((128, NT, L))
            nc.vector._custom_dve(
                SEGMAX2, out=pfo, in0=irb, in1=vall[:, :, 1:], s0=1.0)
            nc.vector.tensor_scalar(
                kkb[:], pfb[:], 1.0, CR, op0=OP.mult, op1=OP.add)
            nc.vector.tensor_scalar(
                t2b[:], kkb[:], CR, 9.5, op0=OP.subtract, op1=OP.add)
            nc.vector.tensor_tensor(
                path[:, :, M - 1], t2b[:], pfb[:], op=OP.subtract)

            # backtrack off the packed P storage: fused one-hot select +
            # running page-max in a single custom-DVE pass (stride-0 out,
            # page-final wins) + 3-op RNE32 decode; all on the DVE
            for t in range(M - 2, -1, -1):
                nxt = path[:, :, t + 1].unsqueeze(2).broadcast_to(
                    (128, NT, L))
                nc.vector._custom_dve(
                    EQSELMAX, out=pfo, in0=ps[:, :, t, :], in1=nxt,
                    s1=float(L))
                nc.vector.tensor_scalar(
                    kkb[:], pfb[:], 1.0, CR, op0=OP.mult, op1=OP.add)
                nc.vector.tensor_scalar(
                    t2b[:], kkb[:], CR, 9.5, op0=OP.subtract, op1=OP.add)
                nc.vector.tensor_tensor(
                    path[:, :, t], t2b[:], pfb[:], op=OP.subtract)

            nc.vector.tensor_copy(pi[:], path[:])
            out_t = out_d.ap().rearrange("(n p) m -> p n m", p=128)
            nc.sync.dma_start(out_t, pi[:])

    nc.compile()
    return nc


_CACHE = {}


def _get_module():
    if "nc" not in _CACHE:
        _CACHE["nc"] = build_module()
    return _CACHE["nc"]


def make_in_maps(X, K, b, W, T):
    AT, TTKP, TTKPP, RST, C0P, BIASD, IRJ, IDN = _consts(X, K, b, W, T)
    consts = {"AT": AT, "TTKP": TTKP, "TTKPP": TTKPP, "RST": RST,
              "C0P": C0P, "BIASD": BIASD, "IRJ": IRJ, "IDN": IDN}
    X = np.ascontiguousarray(X, dtype=np.float32)
    return [dict(consts, XS=X[c * BC:(c + 1) * BC]) for c in range(NCORES)]


def kernel(X, K, b, W, T):
    nc = _get_module()
    in_maps = make_in_maps(X, K, b, W, T)
    res = bass_utils.run_bass_kernel_spmd(nc, in_maps,
                                          core_ids=list(range(NCORES)))
    out = np.concatenate([res.results[c]["OUT"] for c in range(NCORES)],
                         axis=0)
    return out.reshape(B, M, 1).astype(np.int32)


# revision 35
# speedup vs baseline: 1.2440x; 1.2440x over previous
"""CRF decode (conv features -> emission scores -> Viterbi) on 8 TRN2 cores.

Data-parallel over the batch: each core gets B/8 = 4096 words (32 tiles of
128 words on partitions). Per core:

  - conv+emission collapse to one (128 -> 26) linear map A = W @ C applied on
    the PE per 128-word tile (batched: 7 transposes -> one PSUM->SBUF copy,
    14 matmuls into one PSUM bank -> one fused scale+round prep),

  - Viterbi forward DP on packed integers:
      TTKP[p,j,i] = OFF_T + 32*That32[i,j] + (25-i) - 15.5
      vall[p,i]   = OFF_V + 32*vhat[p,i]            (multiple of 32)
      P[p,j]      = max_i (TTKP + vall)             (carries argmax in low bits)
    One custom-DVE scan per (tile, step) computes P directly into persistent
    storage via a stride-0 output AP (page-final element wins), and two
    small Pool ops per (group, step) do the v-update:
      vall' = RNE32(P + 2^28) + s32n   (both adds exact in fp32)
    replacing the 4-op unpack chain of the previous design.  The DVE scan
    at 1 elem/cycle is the architectural roofline here: the Pool engine has
    no max op at all (add/sub/mult only; codegen rejects max/scan/reduce-X)
    and the Act engine is affine-only, so the max cannot be offloaded,

  - emissions/DP overlap via a staggered march of ramped tile-groups,

  - batched backtrack straight off the packed P storage: a hand-patched
    custom-DVE op (EQSELMAX) fuses the one-hot select and the running
    page-max into ONE pass per step, followed by 3 tiny affine decodes,
    all on the DVE so the serial chain has no cross-engine hops,

  - packed final argmax (one SEGMAX2 pass over vall + index table),

  - int32 convert + strided DMA out.

(POOL_START/TTKPP/RST are remnants of an attempted Pool-side max offload,
kept dormant at POOL_START=32: GPSIMD turned out to support no max op.)
"""

import sys

if "/opt/trn_rl_repo" not in sys.path:
    sys.path.insert(0, "/opt/trn_rl_repo")

import numpy as np

import copy as _copy
from dataclasses import dataclass
from typing import Any

import concourse.bacc as bacc
import concourse.mybir as mybir
import concourse.tile as tile
from concourse import bass_utils
from concourse import dve_ops
from concourse.dve_ops import DveOp
from concourse.dve_spec import (
    Spec, Src0, Src1, C0, C1, Idx, SubIdx, eq, lower, scan,
)
from concourse.dve_uop import AluInp, AluOp as UAluOp, DveOpSpec, Trigger


_HAND_CACHE: dict = {}


@dataclass(frozen=True)
class _HandDveOp(DveOp):
    """DveOp whose compiled uops are post-processed by `patch` (adds the
    SUB_DIM_DONE step state that resets an in-body scan at page boundaries —
    not expressible in the Spec language)."""

    patch: Any = None

    def compile(self, ver):
        key = (self.name, ver)
        if (r := _HAND_CACHE.get(key)) is not None:
            return r
        uops = self.patch(lower(self.spec, ver=ver))
        for u in uops:
            u.validate(ver)
        res = DveOpSpec(
            name=self.name, opcode=dve_ops.get_dve_sub_opcode(self.name),
            uops=uops, rd1_en=True)
        _HAND_CACHE[key] = res
        return res


def _reset_scan_stage(dps):
    """In a copied steady datapath, turn the scan combine MAX(CURR, expr)
    into BYPASS(expr): the running max restarts from the current element."""
    for dp in dps:
        if (dp.op in (UAluOp.MAX, UAluOp.ADD)
                and dp.alu_src0 == AluInp.CURR_ALU_OUT):
            dp.op = UAluOp.BYPASS
            dp.alu_src0 = dp.alu_src1
            return True
    raise AssertionError("scan stage not found")


def _patch_segmax(uops):
    assert len(uops) == 2, uops
    seed, steady = uops
    steady.trigger = (Trigger.SRC_TENSOR_DONE, Trigger.SUB_DIM_DONE,
                      Trigger.NONE)
    steady.next_uop = (0, 2, 0)
    step = _copy.deepcopy(steady)
    step.trigger = (Trigger.SRC_TENSOR_DONE, Trigger.SUB_DIM_DONE,
                    Trigger.COUNT)
    step.next_uop = (0, 2, 1)
    step.repeat_count = 1
    _reset_scan_stage(step.datapath_config)
    return [seed, steady, step]


def _register_hand_op(name, spec, patch):
    if name in dve_ops._SUB_OPCODE_FOR_NAME:
        for op in dve_ops.OPS:
            if op.name == name:
                return op
    opcode = max(dve_ops._SUB_OPCODE_FOR_NAME.values()) + 1
    dve_ops._SUB_OPCODE_FOR_NAME[name] = opcode
    shas = {}
    for ver in ("v3", "v4"):
        uops = patch(lower(spec, ver=ver))
        s = DveOpSpec(name=name, opcode=opcode, uops=uops, rd1_en=True)
        shas[ver] = s.sha(ver)
    op = _HandDveOp(name, spec, True, shas, patch=patch)
    dve_ops.OPS.append(op)
    dve_ops.CUSTOM_DVE_SPECS[name] = spec
    return op


def _register_segmax2():
    """r[p,s,n] = running max over n' <= n (within page s) of
    (in0 + in1*s0)[p,s,n'].  Page-final slice [:, :, N-1] is the grouped
    max; with packed-integer operands it carries the argmax in the low bits."""

    def _ref(in0, in1, s0, s1, imm2):
        N = in0.shape[-1]
        P = in0.shape[0]
        a = (np.asarray(in0, np.float32).reshape(P, -1, N)
             + np.asarray(in1, np.float32).reshape(P, -1, N)
             * np.float32(s0))
        return np.maximum.accumulate(a, axis=2).reshape(in0.shape)

    spec = Spec(body=scan(UAluOp.MAX, Src0 + Src1 * C0), reference=_ref)
    return _register_hand_op("SEGMAX2_ANT", spec, _patch_segmax)


SEGMAX2 = _register_segmax2()


def _register_eqsel():
    """out[p,s,n] = (n == in1[p,s,n]) * in0[p,s,n] — one-hot select of a
    row by label index, one pass; max-reduce of the output gives the
    selected (positive) value."""
    name = "EQSEL_ANT"
    if name in dve_ops._SUB_OPCODE_FOR_NAME:
        for op in dve_ops.OPS:
            if op.name == name:
                return op

    def _ref(in0, in1, s0, s1, imm2):
        N = in0.shape[-1]
        P = in0.shape[0]
        a = np.asarray(in0, np.float32).reshape(P, -1, N)
        b = np.asarray(in1, np.float32).reshape(a.shape)
        S = a.shape[1]
        n = (np.arange(S * N, dtype=np.float32)
             - np.repeat(np.arange(S), N) * s1).reshape(S, N)
        return ((n[None] == b).astype(np.float32) * a).reshape(in0.shape)

    spec = Spec(body=eq(Idx - SubIdx * C1, Src1) * Src0, reference=_ref)
    opcode = max(dve_ops._SUB_OPCODE_FOR_NAME.values()) + 1
    dve_ops._SUB_OPCODE_FOR_NAME[name] = opcode
    shas = {}
    for ver in ("v3", "v4"):
        sp = DveOpSpec(name=name, opcode=opcode, uops=lower(spec, ver=ver),
                       rd1_en=True)
        shas[ver] = sp.sha(ver)
    op = DveOp(name, spec, subdim=True, uops_sha=shas)
    dve_ops.OPS.append(op)
    dve_ops.CUSTOM_DVE_SPECS[name] = spec
    return op


EQSEL = _register_eqsel()


def _patch_eqselmax(uops):
    """Fuse a running page-max onto the lowered EQSEL program: the steady
    uop's first BYPASS stage after the select becomes MAX(CURR, select);
    the page-step uop keeps its BYPASS there, which restarts the max at
    page boundaries (same state machine the SubIdx counter already uses)."""
    assert len(uops) == 3, uops
    steady, step = uops[1], uops[2]
    for u in (steady, step):
        dps = u.datapath_config
        last = max(i for i, dp in enumerate(dps) if dp.op != UAluOp.BYPASS)
        assert dps[last].op == UAluOp.MULTIPLY and last + 1 < len(dps), dps
    dp = uops[1].datapath_config[
        1 + max(i for i, d in enumerate(uops[1].datapath_config)
                if d.op != UAluOp.BYPASS)]
    dp.op = UAluOp.MAX
    dp.alu_src0 = AluInp.CURR_ALU_OUT
    dp.alu_src1 = AluInp.PREV_ALU_OUT
    return uops


def _register_eqselmax():
    """r[p,s,n] = running max over n' <= n (within page s) of
    (n' == in1[p,s,n']) * in0[p,s,n'].  Page-final slice is in0 at the
    selected index (in0 > 0 assumed): select + reduce in ONE pass."""

    def _ref(in0, in1, s0, s1, imm2):
        N = in0.shape[-1]
        P = in0.shape[0]
        a = np.asarray(in0, np.float32).reshape(P, -1, N)
        b = np.asarray(in1, np.float32).reshape(a.shape)
        S = a.shape[1]
        n = (np.arange(S * N, dtype=np.float32)
             - np.repeat(np.arange(S), N) * np.float32(s1)).reshape(S, N)
        sel = (n[None] == b).astype(np.float32) * a
        return np.maximum.accumulate(sel, axis=2).reshape(in0.shape)

    spec = Spec(body=eq(Idx - SubIdx * C1, Src1) * Src0, reference=_ref)
    return _register_hand_op("EQSELMAX_ANT", spec, _patch_eqselmax)


EQSELMAX = _register_eqselmax()


F32 = mybir.dt.float32
I32 = mybir.dt.int32
AX = mybir.AxisListType
OP = mybir.AluOpType
ACTF = mybir.ActivationFunctionType

B = 32768
M = 14
H, WD = 16, 8
F = 128
L = 26
KS = 5
NCORES = 8
BC = B // NCORES          # words per core
NT = BC // 128            # 128-word tiles per core (32)

SC = 1024.0               # integer value scale
OFF_T = float(2.0 ** 21)  # packed offsets: P = OFF_T+OFF_V+32*(..) stays
OFF_V = float(2.0 ** 21)  # in (2^21, 2^23) so halves are exact
CR = float(2.0 ** 28)     # RNE-to-multiple-of-32 magnitude (ulp 32 there)

GROUPS = [1, 1, 1, 1, 1, 2, 2, 3, 3, 4, 4, 4, 5]   # word-tiles per stagger slot
GMAX = max(GROUPS)
NG = len(GROUPS)
BASES = [sum(GROUPS[:i]) for i in range(NG)]

POOL_START = 32           # tiles >= this run their max on the Pool engine
RESET = -float(2.0 ** 30)  # page-reset magnitude for the Pool scan trick


def _conv_matrix(K: np.ndarray) -> np.ndarray:
    """C[o, i] such that conv_SAME(x.reshape(H,WD)) flattened == C @ x."""
    K2 = K.reshape(KS, KS).astype(np.float64)
    C = np.zeros((F, F), dtype=np.float64)
    for r in range(H):
        for c in range(WD):
            o = r * WD + c
            for dy in range(KS):
                for dx in range(KS):
                    rr = r + dy - KS // 2
                    cc = c + dx - KS // 2
                    if 0 <= rr < H and 0 <= cc < WD:
                        C[o, rr * WD + cc] = K2[dy, dx]
    return C


def _consts(X, K, b, W, T):
    """Host-side constant tensors (fp64 math, one final fp32 round)."""
    C = _conv_matrix(K)
    A = W.astype(np.float64) @ C                         # (L, F)
    c0 = float(b[0]) * W.astype(np.float64).sum(axis=1)  # (L,)
    Tp = T.astype(np.float64) + c0[None, :]              # T'[i,j] = T[i,j]+c0[j]
    AT = np.ascontiguousarray(A.T).astype(np.float32)    # (F, L)

    # drift estimate from a 256-word exact DP (keeps v centered so the
    # packed range stays well inside (0, 2^23))
    sample = np.ascontiguousarray(X[:256], np.float32)
    sc = (sample.reshape(256 * M, F) @ AT).astype(np.float64)
    sc = sc.reshape(256, M, L)
    v = sc[:, 0] + c0[None, :]
    v0m = v.mean()
    for t in range(1, M):
        v = (v[:, :, None] + Tp[None]).max(axis=1) + sc[:, t]
    d = (v.mean() - v0m) / (M - 1)

    # packed transition table: TTKP[p, j, i] = OFF_T + 32*round(T'[i,j]*SC)
    #                                          + (25 - i) - 15.5
    That = (OFF_T + 32.0 * np.round(Tp.T * SC)
            + (L - 1 - np.arange(L))[None, :] - 15.5)
    TTKP = np.broadcast_to(That.astype(np.float32)[None], (128, L, L)).copy()
    # padded variant for the Pool-engine scan: slot 0 of each 27-wide page
    # holds the RESET sentinel (paired with RST's -2^30 add, it restarts the
    # running max at page boundaries of the single flat tensor_tensor_scan)
    ThatP = np.full((L, L + 1), RESET, np.float64)
    ThatP[:, 1:] = That
    TTKPP = np.broadcast_to(ThatP.astype(np.float32)[None],
                            (128, L, L + 1)).copy()
    RST = np.zeros((128, L * (L + 1)), np.float32)
    RST[:, ::L + 1] = RESET
    # v-init constant: supplies OFF_V and cancels the prep's -CR - OFF_T
    c0pn = (OFF_V + 32.0 * np.round(c0 * SC) + CR + OFF_T).astype(np.float32)
    C0P = np.broadcast_to(c0pn[None], (128, L)).copy()
    # prep bias: psb*(32*SC) + BIASD lands near -2^28 (ulp 32) so the fp32
    # add rounds scores to exact multiples of 32
    biasd = np.full((128, 1), -32.0 * d * SC - CR - OFF_T, np.float32)
    IRJ = np.broadcast_to(
        ((L - 1 - np.arange(L)) - 15.5).astype(np.float32)[None],
        (128, L)).copy()
    IDN = np.eye(128, dtype=np.float32)
    return AT, TTKP, TTKPP, RST, C0P, biasd, IRJ, IDN


def build_module():
    nc = bacc.Bacc("TRN2", target_bir_lowering=False, debug=False,
                   num_devices=NCORES)
    xs = nc.dram_tensor("XS", [BC, M, F], F32, kind="ExternalInput")
    at_d = nc.dram_tensor("AT", [F, L], F32, kind="ExternalInput")
    ttkp_d = nc.dram_tensor("TTKP", [128, L, L], F32, kind="ExternalInput")
    ttkpp_d = nc.dram_tensor("TTKPP", [128, L, L + 1], F32,
                             kind="ExternalInput")
    rst_d = nc.dram_tensor("RST", [128, L * (L + 1)], F32,
                           kind="ExternalInput")
    c0p_d = nc.dram_tensor("C0P", [128, L], F32, kind="ExternalInput")
    bd_d = nc.dram_tensor("BIASD", [128, 1], F32, kind="ExternalInput")
    ir_d = nc.dram_tensor("IRJ", [128, L], F32, kind="ExternalInput")
    id_d = nc.dram_tensor("IDN", [128, 128], F32, kind="ExternalInput")
    out_d = nc.dram_tensor("OUT", [BC, M], I32, kind="ExternalOutput")

    with tile.TileContext(nc) as tc:
        with (
            tc.tile_pool(name="const", bufs=1) as cpool,
            tc.tile_pool(name="pers", bufs=1) as ppool,
            tc.tile_pool(name="xin", bufs=3) as xpool,
            tc.tile_pool(name="xts", bufs=4) as tpool,
            tc.tile_pool(name="qq", bufs=3) as qpool,
            tc.tile_pool(name="psa", bufs=2, space="PSUM") as psA,
            tc.tile_pool(name="psb", bufs=2, space="PSUM") as psB,
        ):
            at = cpool.tile([F, L], F32)
            ttkp = cpool.tile([128, L, L], F32)
            ttkpp = cpool.tile([128, L, L + 1], F32)
            rst = cpool.tile([128, L * (L + 1)], F32)
            c0p = cpool.tile([128, L], F32)
            biasd = cpool.tile([128, 1], F32)
            irj = cpool.tile([128, L], F32)
            idn = cpool.tile([128, 128], F32)
            nc.sync.dma_start(idn[:], id_d.ap())
            nc.scalar.dma_start(at[:], at_d.ap())
            nc.scalar.dma_start(biasd[:], bd_d.ap())
            nc.scalar.dma_start(ttkp[:], ttkp_d.ap())
            nc.scalar.dma_start(ttkpp[:], ttkpp_d.ap())
            nc.scalar.dma_start(rst[:], rst_d.ap())
            nc.scalar.dma_start(c0p[:], c0p_d.ap())
            nc.scalar.dma_start(irj[:], ir_d.ap())

            ps = ppool.tile([128, NT, M - 1, L], F32)   # packed page-finals
            # packed v (multiples of 32); slot 0 is a zero guard column so
            # the Pool scan's padded pages read [guard, v0..v25]
            vall = ppool.tile([128, NT, L + 1], F32)
            s32 = ppool.tile([128, NT, M, L], F32)      # prepped emissions
            path = ppool.tile([128, NT, M], F32)
            pfb = ppool.tile([128, NT], F32)            # selected P / scratch
            kkb = ppool.tile([128, NT], F32)
            t2b = ppool.tile([128, NT], F32)
            pi = ppool.tile([128, NT, M], I32)
            nc.gpsimd.memset(vall[:, :, 0], 0.0)

            xs_t = xs.ap().rearrange("(n p) m f -> n p (m f)", p=128)

            # PE pstate warm-up: ~3us of continuous dummy transposes during
            # the initial DMA wait, so tile 0's real transposes run at full
            # clock instead of PE_CYCLE_PSTATE_LOW (3.7x slower)
            pwarm = psA.tile([128, 128], F32, tag="pwarm")
            for _ in range(12):
                nc.tensor.transpose(pwarm[:], idn[:], idn[:])

            def emit_emission(g):
                base = BASES[g]
                for k in range(GROUPS[g]):
                    wt = base + k
                    xt = xpool.tile([128, M * F], F32, tag="xt")
                    psb = psB.tile([128, M, L], F32, tag="psb")
                    for h in range(2):
                        nc.sync.dma_start(
                            xt[:, h * 7 * F:(h + 1) * 7 * F],
                            xs_t[wt][:, h * 7 * F:(h + 1) * 7 * F])
                        psa = psA.tile([128, 7, 128], F32, tag="psa")
                        for j in range(7):
                            m = h * 7 + j
                            nc.tensor.transpose(
                                psa[:, j, :], xt[:, m * F:(m + 1) * F],
                                idn[:])
                        xh = tpool.tile([128, 7, 128], F32, tag="xts")
                        nc.scalar.activation(xh[:], psa[:], ACTF.Copy)
                        for j in range(7):
                            m = h * 7 + j
                            nc.tensor.matmul(
                                psb[:, m, :], xh[:, j, :], at[:])
                        # fused scale + round-to-multiple-of-32 prep
                        nc.scalar.activation(
                            s32[:, wt, h * 7:(h + 1) * 7, :],
                            psb[:, h * 7:(h + 1) * 7, :], ACTF.Identity,
                            scale=32.0 * SC, bias=biasd[:, 0:1])

            def emit_init(g):
                base, G = BASES[g], GROUPS[g]
                nc.gpsimd.tensor_tensor(
                    vall[:, base:base + G, 1:], s32[:, base:base + G, 0, :],
                    c0p[:].unsqueeze(1).broadcast_to((128, G, L)), op=OP.add)

            def emit_step(g, t):
                base, G = BASES[g], GROUPS[g]
                for k in range(G):
                    wt = base + k
                    pslot = ps[:, wt, t - 1, :]
                    if wt >= POOL_START:
                        # Pool path: materialize q = TTKPP + v over padded
                        # 27-wide pages, then one flat tensor_tensor_scan
                        # whose RST vector restarts the running max at each
                        # page's RESET slot; page-finals land at [:, :, 26]
                        vb = vall[:, wt, :].unsqueeze(1).broadcast_to(
                            (128, L, L + 1))
                        q = qpool.tile([128, L, L + 1], F32, tag="q")
                        nc.gpsimd.tensor_tensor(
                            q[:], ttkpp[:], vb, op=OP.add)
                        nc.gpsimd.tensor_tensor_scan(
                            q[:].rearrange("p a b -> p (a b)"),
                            q[:].rearrange("p a b -> p (a b)"), rst[:],
                            0.0, op0=OP.max, op1=OP.add)
                        nc.gpsimd.tensor_copy(pslot, q[:, :, L])
                        kq = qpool.tile([128, L], F32, tag="kq")
                        nc.gpsimd.tensor_scalar(
                            kq[:], q[:, :, L], 1.0, CR, op0=OP.mult,
                            op1=OP.add)
                        nc.gpsimd.tensor_tensor(
                            vall[:, wt, 1:], kq[:], s32[:, wt, t, :],
                            op=OP.add)
                    else:
                        # stride-0 out: all 26 writes of a page land on the
                        # same address; the page-final (the grouped max) wins
                        vb = vall[:, wt, 1:].unsqueeze(1).broadcast_to(
                            (128, L, L))
                        pout = pslot.unsqueeze(2).broadcast_to((128, L, L))
                        nc.vector._custom_dve(
                            SEGMAX2, out=pout, in0=ttkp[:], in1=vb, s0=1.0)
                # v-update for the DVE tiles of this group:
                # vall = RNE32(P + 2^28) + s32[t]   (both adds exact)
                dhi = min(base + G, POOL_START)
                if dhi > base:
                    kk = qpool.tile([128, GMAX, L], F32, tag="kk")
                    nc.gpsimd.tensor_scalar(
                        kk[:, :dhi - base], ps[:, base:dhi, t - 1, :],
                        1.0, CR, op0=OP.mult, op1=OP.add)
                    nc.gpsimd.tensor_tensor(
                        vall[:, base:dhi, 1:], kk[:, :dhi - base],
                        s32[:, base:dhi, t, :], op=OP.add)

            emit_emission(0)
            emit_emission(1)
            for r in range(M - 1 + NG):
                for g in range(NG):
                    t = r - g
                    if t < 0 or t > M - 1:
                        continue
                    if t == 0:
                        emit_init(g)
                        if g + 2 < NG:
                            emit_emission(g + 2)
                    else:
                        emit_step(g, t)

            # final label: packed argmax of vall + IRJ in one scan pass
            irb = irj[:].unsqueeze(1).broadcast_to((128, NT, L))
            pfo = pfb[:].unsqueeze(2).broadcast_to((128, NT, L))
            nc.vector._custom_dve(
                SEGMAX2, out=pfo, in0=irb, in1=vall[:, :, 1:], s0=1.0)
            nc.vector.tensor_scalar(
                kkb[:], pfb[:], 1.0, CR, op0=OP.mult, op1=OP.add)
            nc.vector.tensor_scalar(
                t2b[:], kkb[:], CR, 9.5, op0=OP.subtract, op1=OP.add)
            nc.vector.tensor_tensor(
                path[:, :, M - 1], t2b[:], pfb[:], op=OP.subtract)

            # backtrack off the packed P storage: fused one-hot select +
            # running page-max in a single custom-DVE pass (stride-0 out,
            # page-final wins) + 3-op RNE32 decode; all on the DVE
            for t in range(M - 2, -1, -1):
                nxt = path[:, :, t + 1].unsqueeze(2).broadcast_to(
                    (128, NT, L))
                nc.vector._custom_dve(
                    EQSELMAX, out=pfo, in0=ps[:, :, t, :], in1=nxt,
                    s1=float(L))
                nc.vector.tensor_scalar(
                    kkb[:], pfb[:], 1.0, CR, op0=OP.mult, op1=OP.add)
                nc.vector.tensor_scalar(
                    t2b[:], kkb[:], CR, 9.5, op0=OP.subtract, op1=OP.add)
                nc.vector.tensor_tensor(
                    path[:, :, t], t2b[:], pfb[:], op=OP.subtract)

            nc.vector.tensor_copy(pi[:], path[:])
            out_t = out_d.ap().rearrange("(n p) m -> p n m", p=128)
            nc.sync.dma_start(out_t, pi[:])

    nc.compile()
    return nc


_CACHE = {}


def _get_module():
    if "nc" not in _CACHE:
        _CACHE["nc"] = build_module()
    return _CACHE["nc"]


def make_in_maps(X, K, b, W, T):
    AT, TTKP, TTKPP, RST, C0P, BIASD, IRJ, IDN = _consts(X, K, b, W, T)
    consts = {"AT": AT, "TTKP": TTKP, "TTKPP": TTKPP, "RST": RST,
              "C0P": C0P, "BIASD": BIASD, "IRJ": IRJ, "IDN": IDN}
    X = np.ascontiguousarray(X, dtype=np.float32)
    return [dict(consts, XS=X[c * BC:(c + 1) * BC]) for c in range(NCORES)]


def kernel(X, K, b, W, T):
    nc = _get_module()
    in_maps = make_in_maps(X, K, b, W, T)
    res = bass_utils.run_bass_kernel_spmd(nc, in_maps,
                                          core_ids=list(range(NCORES)))
    out = np.concatenate([res.results[c]["OUT"] for c in range(NCORES)],
                         axis=0)
    return out.reshape(B, M, 1).astype(np.int32)


# revision 40
# speedup vs baseline: 1.4199x; 1.1414x over previous
"""CRF decode (conv features -> emission scores -> Viterbi) on 8 TRN2 cores.

Data-parallel over the batch: each core gets B/8 = 4096 words (32 tiles of
128 words on partitions). Per core:

  - conv+emission collapse to one (128 -> 26) linear map A = W @ C applied on
    the PE per 128-word tile (batched: 7 transposes -> one PSUM->SBUF copy,
    14 matmuls into one PSUM bank -> one fused scale+round prep),

  - Viterbi forward DP on packed integers:
      TTKP[p,j,i] = OFF_T + 32*That32[i,j] + (25-i) - 15.5
      vall[p,i]   = OFF_V + 32*vhat[p,i]            (multiple of 32)
      P[p,j]      = max_i (TTKP + vall)             (carries argmax in low bits)
    One custom-DVE scan per (tile, step) computes P directly into persistent
    storage via a stride-0 output AP (page-final element wins), and two
    small Pool ops per (group, step) do the v-update:
      vall' = RNE32(P + 2^28) + s32n   (both adds exact in fp32)
    replacing the 4-op unpack chain of the previous design.  The DVE scan
    at 1 elem/cycle is the architectural roofline here: the Pool engine has
    no max op at all (add/sub/mult only; codegen rejects max/scan/reduce-X)
    and the Act engine is affine-only, so the max cannot be offloaded,

  - emissions/DP overlap via a staggered march of ramped tile-groups,

  - batched backtrack straight off the packed P storage: a hand-patched
    custom-DVE op (EQSELMAX) fuses the one-hot select and the running
    page-max into ONE pass per step, followed by 3 tiny affine decodes,
    all on the DVE so the serial chain has no cross-engine hops,

  - packed final argmax (one SEGMAX2 pass over vall + index table),

  - int32 convert + strided DMA out.

(POOL_START/TTKPP/RST are remnants of an attempted Pool-side max offload,
kept dormant at POOL_START=32: GPSIMD turned out to support no max op.)
"""

import sys

if "/opt/trn_rl_repo" not in sys.path:
    sys.path.insert(0, "/opt/trn_rl_repo")

import numpy as np

import copy as _copy
from dataclasses import dataclass
from typing import Any

import concourse.bacc as bacc
import concourse.mybir as mybir
import concourse.tile as tile
from concourse import bass_utils
from concourse import dve_ops
from concourse.dve_ops import DveOp
from concourse.dve_spec import (
    Spec, Src0, Src1, C0, C1, Idx, SubIdx, eq, lower, scan,
)
from concourse.dve_uop import AluInp, AluOp as UAluOp, DveOpSpec, Trigger


_HAND_CACHE: dict = {}


@dataclass(frozen=True)
class _HandDveOp(DveOp):
    """DveOp whose compiled uops are post-processed by `patch` (adds the
    SUB_DIM_DONE step state that resets an in-body scan at page boundaries —
    not expressible in the Spec language)."""

    patch: Any = None

    def compile(self, ver):
        key = (self.name, ver)
        if (r := _HAND_CACHE.get(key)) is not None:
            return r
        uops = self.patch(lower(self.spec, ver=ver))
        for u in uops:
            u.validate(ver)
        res = DveOpSpec(
            name=self.name, opcode=dve_ops.get_dve_sub_opcode(self.name),
            uops=uops, rd1_en=True)
        _HAND_CACHE[key] = res
        return res


def _reset_scan_stage(dps):
    """In a copied steady datapath, turn the scan combine MAX(CURR, expr)
    into BYPASS(expr): the running max restarts from the current element."""
    for dp in dps:
        if (dp.op in (UAluOp.MAX, UAluOp.ADD)
                and dp.alu_src0 == AluInp.CURR_ALU_OUT):
            dp.op = UAluOp.BYPASS
            dp.alu_src0 = dp.alu_src1
            return True
    raise AssertionError("scan stage not found")


def _patch_segmax(uops):
    assert len(uops) == 2, uops
    seed, steady = uops
    steady.trigger = (Trigger.SRC_TENSOR_DONE, Trigger.SUB_DIM_DONE,
                      Trigger.NONE)
    steady.next_uop = (0, 2, 0)
    step = _copy.deepcopy(steady)
    step.trigger = (Trigger.SRC_TENSOR_DONE, Trigger.SUB_DIM_DONE,
                    Trigger.COUNT)
    step.next_uop = (0, 2, 1)
    step.repeat_count = 1
    _reset_scan_stage(step.datapath_config)
    return [seed, steady, step]


def _register_hand_op(name, spec, patch):
    if name in dve_ops._SUB_OPCODE_FOR_NAME:
        for op in dve_ops.OPS:
            if op.name == name:
                return op
    opcode = max(dve_ops._SUB_OPCODE_FOR_NAME.values()) + 1
    dve_ops._SUB_OPCODE_FOR_NAME[name] = opcode
    shas = {}
    for ver in ("v3", "v4"):
        uops = patch(lower(spec, ver=ver))
        s = DveOpSpec(name=name, opcode=opcode, uops=uops, rd1_en=True)
        shas[ver] = s.sha(ver)
    op = _HandDveOp(name, spec, True, shas, patch=patch)
    dve_ops.OPS.append(op)
    dve_ops.CUSTOM_DVE_SPECS[name] = spec
    return op


def _register_segmax2():
    """r[p,s,n] = running max over n' <= n (within page s) of
    (in0 + in1*s0)[p,s,n'].  Page-final slice [:, :, N-1] is the grouped
    max; with packed-integer operands it carries the argmax in the low bits."""

    def _ref(in0, in1, s0, s1, imm2):
        N = in0.shape[-1]
        P = in0.shape[0]
        a = (np.asarray(in0, np.float32).reshape(P, -1, N)
             + np.asarray(in1, np.float32).reshape(P, -1, N)
             * np.float32(s0))
        return np.maximum.accumulate(a, axis=2).reshape(in0.shape)

    spec = Spec(body=scan(UAluOp.MAX, Src0 + Src1 * C0), reference=_ref)
    return _register_hand_op("SEGMAX2_ANT", spec, _patch_segmax)


SEGMAX2 = _register_segmax2()


def _register_eqsel():
    """out[p,s,n] = (n == in1[p,s,n]) * in0[p,s,n] — one-hot select of a
    row by label index, one pass; max-reduce of the output gives the
    selected (positive) value."""
    name = "EQSEL_ANT"
    if name in dve_ops._SUB_OPCODE_FOR_NAME:
        for op in dve_ops.OPS:
            if op.name == name:
                return op

    def _ref(in0, in1, s0, s1, imm2):
        N = in0.shape[-1]
        P = in0.shape[0]
        a = np.asarray(in0, np.float32).reshape(P, -1, N)
        b = np.asarray(in1, np.float32).reshape(a.shape)
        S = a.shape[1]
        n = (np.arange(S * N, dtype=np.float32)
             - np.repeat(np.arange(S), N) * s1).reshape(S, N)
        return ((n[None] == b).astype(np.float32) * a).reshape(in0.shape)

    spec = Spec(body=eq(Idx - SubIdx * C1, Src1) * Src0, reference=_ref)
    opcode = max(dve_ops._SUB_OPCODE_FOR_NAME.values()) + 1
    dve_ops._SUB_OPCODE_FOR_NAME[name] = opcode
    shas = {}
    for ver in ("v3", "v4"):
        sp = DveOpSpec(name=name, opcode=opcode, uops=lower(spec, ver=ver),
                       rd1_en=True)
        shas[ver] = sp.sha(ver)
    op = DveOp(name, spec, subdim=True, uops_sha=shas)
    dve_ops.OPS.append(op)
    dve_ops.CUSTOM_DVE_SPECS[name] = spec
    return op


EQSEL = _register_eqsel()


def _patch_eqselmax(uops):
    """Fuse a running page-max onto the lowered EQSEL program: the steady
    uop's first BYPASS stage after the select becomes MAX(CURR, select);
    the page-step uop keeps its BYPASS there, which restarts the max at
    page boundaries (same state machine the SubIdx counter already uses)."""
    assert len(uops) == 3, uops
    steady, step = uops[1], uops[2]
    for u in (steady, step):
        dps = u.datapath_config
        last = max(i for i, dp in enumerate(dps) if dp.op != UAluOp.BYPASS)
        assert dps[last].op == UAluOp.MULTIPLY and last + 1 < len(dps), dps
    dp = uops[1].datapath_config[
        1 + max(i for i, d in enumerate(uops[1].datapath_config)
                if d.op != UAluOp.BYPASS)]
    dp.op = UAluOp.MAX
    dp.alu_src0 = AluInp.CURR_ALU_OUT
    dp.alu_src1 = AluInp.PREV_ALU_OUT
    return uops


def _register_eqselmax():
    """r[p,s,n] = running max over n' <= n (within page s) of
    (n' == in1[p,s,n']) * in0[p,s,n'].  Page-final slice is in0 at the
    selected index (in0 > 0 assumed): select + reduce in ONE pass."""

    def _ref(in0, in1, s0, s1, imm2):
        N = in0.shape[-1]
        P = in0.shape[0]
        a = np.asarray(in0, np.float32).reshape(P, -1, N)
        b = np.asarray(in1, np.float32).reshape(a.shape)
        S = a.shape[1]
        n = (np.arange(S * N, dtype=np.float32)
             - np.repeat(np.arange(S), N) * np.float32(s1)).reshape(S, N)
        sel = (n[None] == b).astype(np.float32) * a
        return np.maximum.accumulate(sel, axis=2).reshape(in0.shape)

    spec = Spec(body=eq(Idx - SubIdx * C1, Src1) * Src0, reference=_ref)
    return _register_hand_op("EQSELMAX_ANT", spec, _patch_eqselmax)


EQSELMAX = _register_eqselmax()


F32 = mybir.dt.float32
I32 = mybir.dt.int32
AX = mybir.AxisListType
OP = mybir.AluOpType
ACTF = mybir.ActivationFunctionType

B = 32768
M = 14
H, WD = 16, 8
F = 128
L = 26
KS = 5
NCORES = 8
BC = B // NCORES          # words per core
NT = BC // 128            # 128-word tiles per core (32)

SC = 1024.0               # integer value scale
OFF_T = float(2.0 ** 21)  # packed offsets: P = OFF_T+OFF_V+32*(..) stays
OFF_V = float(2.0 ** 21)  # in (2^21, 2^23) so halves are exact
CR = float(2.0 ** 28)     # RNE-to-multiple-of-32 magnitude (ulp 32 there)

GROUPS = [1, 1, 1, 1, 1, 2, 2, 3, 3, 4, 4, 4, 5]   # word-tiles per stagger slot
GMAX = max(GROUPS)
NG = len(GROUPS)
BASES = [sum(GROUPS[:i]) for i in range(NG)]

POOL_START = 32           # tiles >= this run their max on the Pool engine
RESET = -float(2.0 ** 30)  # page-reset magnitude for the Pool scan trick


def _conv_matrix(K: np.ndarray) -> np.ndarray:
    """C[o, i] such that conv_SAME(x.reshape(H,WD)) flattened == C @ x."""
    K2 = K.reshape(KS, KS).astype(np.float64)
    C = np.zeros((F, F), dtype=np.float64)
    for r in range(H):
        for c in range(WD):
            o = r * WD + c
            for dy in range(KS):
                for dx in range(KS):
                    rr = r + dy - KS // 2
                    cc = c + dx - KS // 2
                    if 0 <= rr < H and 0 <= cc < WD:
                        C[o, rr * WD + cc] = K2[dy, dx]
    return C


def _consts(X, K, b, W, T):
    """Host-side constant tensors (fp64 math, one final fp32 round)."""
    C = _conv_matrix(K)
    A = W.astype(np.float64) @ C                         # (L, F)
    c0 = float(b[0]) * W.astype(np.float64).sum(axis=1)  # (L,)
    Tp = T.astype(np.float64) + c0[None, :]              # T'[i,j] = T[i,j]+c0[j]
    AT = np.ascontiguousarray(A.T).astype(np.float32)    # (F, L)

    # drift estimate from a 256-word exact DP (keeps v centered so the
    # packed range stays well inside (0, 2^23))
    sample = np.ascontiguousarray(X[:256], np.float32)
    sc = (sample.reshape(256 * M, F) @ AT).astype(np.float64)
    sc = sc.reshape(256, M, L)
    v = sc[:, 0] + c0[None, :]
    v0m = v.mean()
    for t in range(1, M):
        v = (v[:, :, None] + Tp[None]).max(axis=1) + sc[:, t]
    d = (v.mean() - v0m) / (M - 1)

    # packed transition table: TTKP[p, j, i] = OFF_T + 32*round(T'[i,j]*SC)
    #                                          + (25 - i) - 15.5
    That = (OFF_T + 32.0 * np.round(Tp.T * SC)
            + (L - 1 - np.arange(L))[None, :] - 15.5)
    TTKP = np.broadcast_to(That.astype(np.float32)[None], (128, L, L)).copy()
    # padded variant for the Pool-engine scan: slot 0 of each 27-wide page
    # holds the RESET sentinel (paired with RST's -2^30 add, it restarts the
    # running max at page boundaries of the single flat tensor_tensor_scan)
    ThatP = np.full((L, L + 1), RESET, np.float64)
    ThatP[:, 1:] = That
    TTKPP = np.broadcast_to(ThatP.astype(np.float32)[None],
                            (128, L, L + 1)).copy()
    RST = np.zeros((128, L * (L + 1)), np.float32)
    RST[:, ::L + 1] = RESET
    # v-init constant: supplies OFF_V and cancels the prep's -CR - OFF_T
    c0pn = (OFF_V + 32.0 * np.round(c0 * SC) + CR + OFF_T).astype(np.float32)
    C0P = np.broadcast_to(c0pn[None], (128, L)).copy()
    # prep bias: psb*(32*SC) + BIASD lands near -2^28 (ulp 32) so the fp32
    # add rounds scores to exact multiples of 32
    biasd = np.full((128, 1), -32.0 * d * SC - CR - OFF_T, np.float32)
    IRJ = np.broadcast_to(
        ((L - 1 - np.arange(L)) - 15.5).astype(np.float32)[None],
        (128, L)).copy()
    IDN = np.eye(128, dtype=np.float32)
    return AT, TTKP, TTKPP, RST, C0P, biasd, IRJ, IDN


def build_module():
    nc = bacc.Bacc("TRN2", target_bir_lowering=False, debug=False,
                   num_devices=NCORES)
    xs = nc.dram_tensor("XS", [BC, M, F], F32, kind="ExternalInput")
    at_d = nc.dram_tensor("AT", [F, L], F32, kind="ExternalInput")
    ttkp_d = nc.dram_tensor("TTKP", [128, L, L], F32, kind="ExternalInput")
    ttkpp_d = nc.dram_tensor("TTKPP", [128, L, L + 1], F32,
                             kind="ExternalInput")
    rst_d = nc.dram_tensor("RST", [128, L * (L + 1)], F32,
                           kind="ExternalInput")
    c0p_d = nc.dram_tensor("C0P", [128, L], F32, kind="ExternalInput")
    bd_d = nc.dram_tensor("BIASD", [128, 1], F32, kind="ExternalInput")
    ir_d = nc.dram_tensor("IRJ", [128, L], F32, kind="ExternalInput")
    id_d = nc.dram_tensor("IDN", [128, 128], F32, kind="ExternalInput")
    out_d = nc.dram_tensor("OUT", [BC, M], I32, kind="ExternalOutput")

    with tile.TileContext(nc) as tc:
        with (
            tc.tile_pool(name="const", bufs=1) as cpool,
            tc.tile_pool(name="pers", bufs=1) as ppool,
            tc.tile_pool(name="xin", bufs=3) as xpool,
            tc.tile_pool(name="xts", bufs=4) as tpool,
            tc.tile_pool(name="qq", bufs=3) as qpool,
            tc.tile_pool(name="psa", bufs=2, space="PSUM") as psA,
            tc.tile_pool(name="psb", bufs=2, space="PSUM") as psB,
        ):
            at = cpool.tile([F, L], F32)
            ttkp = cpool.tile([128, L, L], F32)
            ttkpp = cpool.tile([128, L, L + 1], F32)
            rst = cpool.tile([128, L * (L + 1)], F32)
            c0p = cpool.tile([128, L], F32)
            biasd = cpool.tile([128, 1], F32)
            irj = cpool.tile([128, L], F32)
            idn = cpool.tile([128, 128], F32)
            # const DMAs ride the Pool queue (idle at startup) and the sync
            # queue — NOT the Act queue, whose sequencer must stay free for
            # the first tile's copy/prep chain
            nc.sync.dma_start(idn[:], id_d.ap())
            nc.gpsimd.dma_start(at[:], at_d.ap())
            nc.gpsimd.dma_start(biasd[:], bd_d.ap())
            nc.gpsimd.dma_start(c0p[:], c0p_d.ap())
            nc.gpsimd.dma_start(irj[:], ir_d.ap())
            nc.gpsimd.dma_start(ttkpp[:], ttkpp_d.ap())
            nc.gpsimd.dma_start(rst[:], rst_d.ap())

            ps = ppool.tile([128, NT, M - 1, L], F32)   # packed page-finals
            # packed v (multiples of 32); slot 0 is a zero guard column so
            # the Pool scan's padded pages read [guard, v0..v25]
            vall = ppool.tile([128, NT, L + 1], F32)
            s32 = ppool.tile([128, NT, M, L], F32)      # prepped emissions
            path = ppool.tile([128, NT, M], F32)
            pfb = ppool.tile([128, NT], F32)            # selected P / scratch
            kkb = ppool.tile([128, NT], F32)
            t2b = ppool.tile([128, NT], F32)
            pi = ppool.tile([128, NT, M], I32)
            nc.gpsimd.memset(vall[:, :, 0], 0.0)

            xs_t = xs.ap().rearrange("(n p) m f -> n p (m f)", p=128)

            # fast path for the very first scan: DMA/transpose/matmul/prep
            # ONLY (tile 0, m=0) ahead of everything else, so the group-0
            # init (which needs just s32[:,0,0,:]) unblocks the DVE ~7us
            # earlier than waiting for tile 0's full 7-letter half
            xt0 = xpool.tile([128, M * F], F32, tag="xt")
            psb0 = psB.tile([128, M, L], F32, tag="psb")
            nc.sync.dma_start(xt0[:, 0:F], xs_t[0][:, 0:F])
            nc.sync.dma_start(ttkp[:], ttkp_d.ap())
            psa0 = psA.tile([128, 7, 128], F32, tag="psa")
            nc.tensor.transpose(psa0[:, 0, :], xt0[:, 0:F], idn[:])
            xh0 = tpool.tile([128, 7, 128], F32, tag="xts")
            nc.scalar.activation(xh0[:, 0, :], psa0[:, 0, :], ACTF.Copy)
            nc.tensor.matmul(psb0[:, 0, :], xh0[:, 0, :], at[:])
            nc.scalar.activation(
                s32[:, 0, 0:1, :], psb0[:, 0:1, :], ACTF.Identity,
                scale=32.0 * SC, bias=biasd[:, 0:1])

            # PE pstate warm-up: ~3us of continuous dummy transposes during
            # the initial DMA wait, so tile 0's real transposes run at full
            # clock instead of PE_CYCLE_PSTATE_LOW (3.7x slower)
            pwarm = psA.tile([128, 128], F32, tag="pwarm")
            for _ in range(12):
                nc.tensor.transpose(pwarm[:], idn[:], idn[:])

            def emit_emission(g):
                base = BASES[g]
                for k in range(GROUPS[g]):
                    wt = base + k
                    xt = xpool.tile([128, M * F], F32, tag="xt")
                    psb = psB.tile([128, M, L], F32, tag="psb")
                    for h in range(2):
                        # tile 0's m=0 was done by the fast path above
                        mlo = 1 if (wt == 0 and h == 0) else h * 7
                        nc.sync.dma_start(
                            xt[:, mlo * F:(h + 1) * 7 * F],
                            xs_t[wt][:, mlo * F:(h + 1) * 7 * F])
                        psa = psA.tile([128, 7, 128], F32, tag="psa")
                        jlo = mlo - h * 7
                        for m in range(mlo, (h + 1) * 7):
                            j = m - h * 7
                            nc.tensor.transpose(
                                psa[:, j, :], xt[:, m * F:(m + 1) * F],
                                idn[:])
                        xh = tpool.tile([128, 7, 128], F32, tag="xts")
                        nc.scalar.activation(
                            xh[:, jlo:, :], psa[:, jlo:, :], ACTF.Copy)
                        for m in range(mlo, (h + 1) * 7):
                            j = m - h * 7
                            nc.tensor.matmul(
                                psb[:, m, :], xh[:, j, :], at[:])
                        # fused scale + round-to-multiple-of-32 prep
                        nc.scalar.activation(
                            s32[:, wt, mlo:(h + 1) * 7, :],
                            psb[:, mlo:(h + 1) * 7, :], ACTF.Identity,
                            scale=32.0 * SC, bias=biasd[:, 0:1])

            def emit_init(g):
                base, G = BASES[g], GROUPS[g]
                nc.gpsimd.tensor_tensor(
                    vall[:, base:base + G, 1:], s32[:, base:base + G, 0, :],
                    c0p[:].unsqueeze(1).broadcast_to((128, G, L)), op=OP.add)

            def emit_step(g, t):
                base, G = BASES[g], GROUPS[g]
                for k in range(G):
                    wt = base + k
                    pslot = ps[:, wt, t - 1, :]
                    if wt >= POOL_START:
                        # Pool path: materialize q = TTKPP + v over padded
                        # 27-wide pages, then one flat tensor_tensor_scan
                        # whose RST vector restarts the running max at each
                        # page's RESET slot; page-finals land at [:, :, 26]
                        vb = vall[:, wt, :].unsqueeze(1).broadcast_to(
                            (128, L, L + 1))
                        q = qpool.tile([128, L, L + 1], F32, tag="q")
                        nc.gpsimd.tensor_tensor(
                            q[:], ttkpp[:], vb, op=OP.add)
                        nc.gpsimd.tensor_tensor_scan(
                            q[:].rearrange("p a b -> p (a b)"),
                            q[:].rearrange("p a b -> p (a b)"), rst[:],
                            0.0, op0=OP.max, op1=OP.add)
                        nc.gpsimd.tensor_copy(pslot, q[:, :, L])
                        kq = qpool.tile([128, L], F32, tag="kq")
                        nc.gpsimd.tensor_scalar(
                            kq[:], q[:, :, L], 1.0, CR, op0=OP.mult,
                            op1=OP.add)
                        nc.gpsimd.tensor_tensor(
                            vall[:, wt, 1:], kq[:], s32[:, wt, t, :],
                            op=OP.add)
                    else:
                        # stride-0 out: all 26 writes of a page land on the
                        # same address; the page-final (the grouped max) wins
                        vb = vall[:, wt, 1:].unsqueeze(1).broadcast_to(
                            (128, L, L))
                        pout = pslot.unsqueeze(2).broadcast_to((128, L, L))
                        nc.vector._custom_dve(
                            SEGMAX2, out=pout, in0=ttkp[:], in1=vb, s0=1.0)
                # v-update for the DVE tiles of this group:
                # vall = RNE32(P + 2^28) + s32[t]   (both adds exact)
                dhi = min(base + G, POOL_START)
                if dhi > base:
                    kk = qpool.tile([128, GMAX, L], F32, tag="kk")
                    nc.gpsimd.tensor_scalar(
                        kk[:, :dhi - base], ps[:, base:dhi, t - 1, :],
                        1.0, CR, op0=OP.mult, op1=OP.add)
                    nc.gpsimd.tensor_tensor(
                        vall[:, base:dhi, 1:], kk[:, :dhi - base],
                        s32[:, base:dhi, t, :], op=OP.add)

            emit_emission(0)
            emit_emission(1)
            for r in range(M - 1 + NG):
                for g in range(NG):
                    t = r - g
                    if t < 0 or t > M - 1:
                        continue
                    if t == 0:
                        emit_init(g)
                        if g + 2 < NG:
                            emit_emission(g + 2)
                    else:
                        emit_step(g, t)

            # final label: packed argmax of vall + IRJ in one scan pass
            irb = irj[:].unsqueeze(1).broadcast_to((128, NT, L))
            pfo = pfb[:].unsqueeze(2).broadcast_to((128, NT, L))
            nc.vector._custom_dve(
                SEGMAX2, out=pfo, in0=irb, in1=vall[:, :, 1:], s0=1.0)
            nc.vector.tensor_scalar(
                kkb[:], pfb[:], 1.0, CR, op0=OP.mult, op1=OP.add)
            nc.vector.tensor_scalar(
                t2b[:], kkb[:], CR, 9.5, op0=OP.subtract, op1=OP.add)
            nc.vector.tensor_tensor(
                path[:, :, M - 1], t2b[:], pfb[:], op=OP.subtract)

            # backtrack off the packed P storage: fused one-hot select +
            # running page-max in a single custom-DVE pass (stride-0 out,
            # page-final wins) + 3-op RNE32 decode; all on the DVE
            for t in range(M - 2, -1, -1):
                nxt = path[:, :, t + 1].unsqueeze(2).broadcast_to(
                    (128, NT, L))
                nc.vector._custom_dve(
                    EQSELMAX, out=pfo, in0=ps[:, :, t, :], in1=nxt,
                    s1=float(L))
                nc.vector.tensor_scalar(
                    kkb[:], pfb[:], 1.0, CR, op0=OP.mult, op1=OP.add)
                nc.vector.tensor_scalar(
                    t2b[:], kkb[:], CR, 9.5, op0=OP.subtract, op1=OP.add)
                nc.vector.tensor_tensor(
                    path[:, :, t], t2b[:], pfb[:], op=OP.subtract)

            nc.vector.tensor_copy(pi[:], path[:])
            out_t = out_d.ap().rearrange("(n p) m -> p n m", p=128)
            nc.sync.dma_start(out_t, pi[:])

    nc.compile()
    return nc


_CACHE = {}


def _get_module():
    if "nc" not in _CACHE:
        _CACHE["nc"] = build_module()
    return _CACHE["nc"]


def make_in_maps(X, K, b, W, T):
    AT, TTKP, TTKPP, RST, C0P, BIASD, IRJ, IDN = _consts(X, K, b, W, T)
    consts = {"AT": AT, "TTKP": TTKP, "TTKPP": TTKPP, "RST": RST,
              "C0P": C0P, "BIASD": BIASD, "IRJ": IRJ, "IDN": IDN}
    X = np.ascontiguousarray(X, dtype=np.float32)
    return [dict(consts, XS=X[c * BC:(c + 1) * BC]) for c in range(NCORES)]


def kernel(X, K, b, W, T):
    nc = _get_module()
    in_maps = make_in_maps(X, K, b, W, T)
    res = bass_utils.run_bass_kernel_spmd(nc, in_maps,
                                          core_ids=list(range(NCORES)))
    out = np.concatenate([res.results[c]["OUT"] for c in range(NCORES)],
                         axis=0)
    return out.reshape(B, M, 1).astype(np.int32)
